# revision 23
# baseline (speedup 1.0000x reference)
"""Chamfer distance loss kernel for Trainium2 (8 NeuronCores).

Problem: points1, points2 [8, 4096, 3] fp32 -> scalar loss.
Sharding: data-parallel over batch; core b handles batch b. Host averages the
8 per-batch losses.

Per-core algorithm:
  dist[i,j] = n1[i] + n2[j] - 2*x1[i].x2[j]  (squared L2)
  * TensorE: PSUM[i,j] = sum_k L[k,i]*R[k,j] where the 21 live rows are a
    3-level bf16 split of the coordinates (hi/lo/lo2) plus rows carrying
    -n_j/2 (3-level bf16 split), so PSUM = (x_i.x_j)_fp32ish - n_j/2.
    The 21 operand rows (zero-padded to 32) are replicated at partition
    bases 0/32/64/96 and the PE is driven in 4x-row-tiling mode
    (tile_position=(32q, 0)): four K=21 matmuls execute concurrently in
    separate 32-row tiles.  The operand buffers (including the splits,
    norm rows, padding and quadrant replicas) are assembled on the HOST
    and DMAed in directly -- layout/precision prep is O(N), all O(N^2)
    work stays on device -- which removes the on-chip setup phase from
    the critical path.
    Three additional rows carry -n_i/2 (L-side norm splits against R-side
    ones), so PSUM holds -dist/2 directly and no per-row ACT bias is needed.
  * Reduction (engine-balanced; the kernel is ScalarE/VectorE-bound and both
    run ~1 elem/lane/cycle, so work is split to equalize them):
      A-unit (44 of 64): ScalarE ACT Copy psum -> fp16 (-dist/2) for both
        2048-halves, then one VectorE fp16 max-tree (2x mode) over
        [128,4096] + reduce_max -> MB[:, col].
      M-unit (20 of 64): half h0 is reduced STRAIGHT off PSUM by VectorE
        tensor_reduce(max) -> MA[:, col]; half h1 goes through the ScalarE
        cast + a fp16 half-tree -> MB[:, col].  M-units keep ScalarE fed
        (PSUM banks free after 2.3us instead of 4.6us) while soaking up
        VectorE slack.
  * RMAX = max(MA, MB) (same -dist/2 scale); means via ones-vector matmul
    partition-sum scaled by -2/4096.
"""

import numpy as np

N = 4096          # points per cloud
P = 128           # partitions
TT = N // P       # 32 column blocks
D3 = 3
JB = 512          # matmul moving free dim
HALF = N // 2     # per-PSUM-allocation j extent (4 banks)
B = 8             # batches / cores
KPAD = 128
NROWS = 24        # live rows: 18 coord-pair rows + 3 R-norm rows + 3 L-norm rows
N_MIX = 32        # of the 64 (direction, i-tile) units, how many are M-units
NEG_INIT = -1.0e30

# (L-level, R-level) pairs for the 3-level bf16 product expansion.
COORD_PAIRS = [(0, 0), (0, 1), (0, 2), (1, 0), (1, 1), (2, 0)]

_NC_CACHE = {}


def _build_nc():
    import concourse.bacc as bacc
    import concourse.tile as tile
    from concourse import mybir

    FP32 = mybir.dt.float32
    BF16 = mybir.dt.bfloat16

    nc = bacc.Bacc("TRN2", target_bir_lowering=False, debug=False)
    bufs_in = {}
    for name in ("lbuf0", "rbuf1", "lbuf1", "rbuf0"):
        bufs_in[name] = nc.dram_tensor(name, [KPAD, N], BF16,
                                       kind="ExternalInput").ap()
    out = nc.dram_tensor("loss", [1, 1], FP32, kind="ExternalOutput").ap()

    with tile.TileContext(nc) as tc:
        _emit(tc, bufs_in, out)

    nc.compile()
    return nc


def _emit(tc, bufs_in, out):
    import concourse.bass as bass  # noqa: F401
    from concourse import mybir

    FP32 = mybir.dt.float32
    BF16 = mybir.dt.bfloat16
    FP16 = mybir.dt.float16
    AX = mybir.AxisListType
    OP = mybir.AluOpType
    AF = mybir.ActivationFunctionType

    nc = tc.nc

    # Which (direction, i-tile) units are M-units, spread evenly and
    # INCLUDING both endpoints: an M-unit first lets VectorE start its PSUM
    # reduce right after the first matmul group (no wait for two ScalarE
    # casts), and an M-unit last shortens the drain tail.
    n_units = 2 * TT
    mix = {(k * (n_units - 1)) // (N_MIX - 1) for k in range(N_MIX)}

    from contextlib import ExitStack
    with ExitStack() as ctx:
        consts = ctx.enter_context(tc.tile_pool(name="consts", bufs=1))

        ones_col = consts.tile([P, 1], FP32, name="ones_col", tag="ones_col")
        nc.vector.memset(ones_col, 1.0)

        Lbufs, Rbufs = [], []
        for m in range(2):
            Lb = consts.tile([KPAD, N], BF16, name=f"Lbuf{m}", tag=f"Lbuf{m}")
            Rb = consts.tile([KPAD, N], BF16, name=f"Rbuf{m}", tag=f"Rbuf{m}")
            Lbufs.append(Lb)
            Rbufs.append(Rb)
        MA = consts.tile([P, 2 * TT], FP32, name="MA", tag="MA")
        MB = consts.tile([P, 2 * TT], FP32, name="MB", tag="MB")
        nc.vector.memset(MA, NEG_INIT)

        # HAM warm-up: ~4.3us of back-to-back dummy matmuls while the operand
        # DMAs are in flight.  The PE's activity monitor only unthrottles
        # (1.2 -> 2.4 GHz) after a full 4096-cycle busy window; the main loop
        # alone never fills one (~30% duty), but once warm the <=1.5us gaps
        # between MM groups keep it warm.  Cold matmuls cost 596ns vs ~230ns
        # per warm 4x-tiled group, and the MM turnaround sits on ScalarE's
        # critical path at every PSUM slot handoff.
        warm = consts.tile([32, JB], BF16, name="warm", tag="warm")
        nc.vector.memset(warm, 0.0)

        # Operand loads, chunked so the first units' semaphores fire early
        # (a whole-buffer DMA only signals at full completion ~10us in).
        # Rbuf1 (every unit j-scans all of it) is split 4+4 across the two
        # HWDGE queues; Lbuf0's first chunk covers i-tiles 0..7.
        CH = 512
        for c in (0, 2):
            nc.sync.dma_start(out=Rbufs[1][:, c * CH:(c + 1) * CH],
                              in_=bufs_in["rbuf1"][:, c * CH:(c + 1) * CH])
        for c in (1, 3):
            nc.scalar.dma_start(out=Rbufs[1][:, c * CH:(c + 1) * CH],
                                in_=bufs_in["rbuf1"][:, c * CH:(c + 1) * CH])
        nc.scalar.dma_start(out=Lbufs[0][:, 0:1024],
                            in_=bufs_in["lbuf0"][:, 0:1024])
        for c in range(4, 8):
            (nc.sync if c % 2 else nc.scalar).dma_start(
                out=Rbufs[1][:, c * CH:(c + 1) * CH],
                in_=bufs_in["rbuf1"][:, c * CH:(c + 1) * CH])
        nc.sync.dma_start(out=Lbufs[0][:, 1024:N],
                          in_=bufs_in["lbuf0"][:, 1024:N])
        nc.sync.dma_start(out=Lbufs[1], in_=bufs_in["lbuf1"])
        nc.sync.dma_start(out=Rbufs[0], in_=bufs_in["rbuf0"])

        with tc.tile_pool(name="wps", bufs=1, space="PSUM") as wps:
            wt = wps.tile([P, JB], FP32, name="wt", tag="wt")
            for _ in range(10):
                nc.tensor.matmul(wt, lhsT=warm[:, 0:P], rhs=warm,
                                 start=True, stop=True)

        # ---------------- main loop ----------------
        unit = 0
        with tc.tile_pool(name="psm", bufs=2, space="PSUM") as psm, \
             tc.tile_pool(name="dpool", bufs=2) as dpool, \
             tc.tile_pool(name="mpool", bufs=2) as mpool, \
             tc.tile_pool(name="papool", bufs=2) as papool, \
             tc.tile_pool(name="pbpool", bufs=2) as pbpool, \
             tc.tile_pool(name="mapool", bufs=2) as mapool, \
             tc.tile_pool(name="mbpool", bufs=2) as mbpool:
            def mm_group(ps, Lb, Rb, t, h):
                for u in range(HALF // JB):
                    j0 = h * HALF + u * JB
                    q = 32 * (u % 4)
                    nc.tensor.matmul(
                        ps[:, u * JB:(u + 1) * JB],
                        lhsT=Lb[q:q + NROWS, t * P:(t + 1) * P],
                        rhs=Rb[q:q + NROWS, j0:j0 + JB],
                        start=True, stop=True,
                        tile_position=(q, 0),
                    )

            for d in range(2):
                Lb = Lbufs[0] if d == 0 else Lbufs[1]
                Rb = Rbufs[1] if d == 0 else Rbufs[0]
                for t in range(TT):
                    col = d * TT + t
                    use_m = unit in mix
                    unit += 1
                    if use_m:
                        # --- M-unit: h0 straight off PSUM, h1 cast+half-tree
                        # TR half first: the PSUM reduce starts while ScalarE
                        # still casts the previous unit, keeping VectorE fed.
                        late = True
                        if late:
                            ps0 = psm.tile([P, HALF], FP32, name="ps", tag="ps")
                            mm_group(ps0, Lb, Rb, t, 0)
                            nc.vector.tensor_reduce(
                                out=MA[:, col:col + 1], in_=ps0,
                                axis=AX.X, op=OP.max)
                        ps1 = psm.tile([P, HALF], FP32, name="ps", tag="ps")
                        mm_group(ps1, Lb, Rb, t, 1)
                        Dm = mpool.tile([P, HALF], FP16, name="Dm", tag="Dm")
                        nc.scalar.copy(Dm, ps1)
                        if not late:
                            ps0 = psm.tile([P, HALF], FP32, name="ps", tag="ps")
                            mm_group(ps0, Lb, Rb, t, 0)
                            nc.vector.tensor_reduce(
                                out=MA[:, col:col + 1], in_=ps0,
                                axis=AX.X, op=OP.max)
                        ma = mapool.tile([P, HALF // 2], FP16, name="ma",
                                         tag="ma")
                        mb = mbpool.tile([P, HALF // 4], FP16, name="mb",
                                         tag="mb")
                        nc.vector.tensor_max(ma, Dm[:, :1024], Dm[:, 1024:])
                        nc.vector.tensor_max(mb, ma[:, :512], ma[:, 512:1024])
                        nc.vector.tensor_reduce(
                            out=MB[:, col:col + 1],
                            in_=mb, axis=AX.X, op=OP.max)
                    else:
                        # --- A-unit: ScalarE cast both halves + fp16 tree ---
                        Dt = dpool.tile([P, N], FP16, name="Dt", tag="Dt")
                        for h in range(2):
                            ps = psm.tile([P, HALF], FP32, name="ps", tag="ps")
                            mm_group(ps, Lb, Rb, t, h)
                            nc.scalar.copy(Dt[:, h * HALF:(h + 1) * HALF], ps)
                        PA = papool.tile([P, HALF], FP16, name="PA", tag="PA")
                        PB = pbpool.tile([P, HALF // 2], FP16,
                                         name="PB", tag="PB")
                        nc.vector.tensor_max(PA, Dt[:, :HALF], Dt[:, HALF:])
                        nc.vector.tensor_max(PB, PA[:, :1024], PA[:, 1024:2048])
                        nc.vector.tensor_max(PA[:, :512], PB[:, :512],
                                             PB[:, 512:1024])
                        nc.vector.tensor_reduce(
                            out=MB[:, col:col + 1],
                            in_=PA[:, :512], axis=AX.X, op=OP.max,
                        )

        # ---------------- fixup + final reduction ----------------
        with tc.tile_pool(name="psf", bufs=1, space="PSUM") as psf, \
             tc.tile_pool(name="ftmp", bufs=1) as ftmp:
            RMAX = ftmp.tile([P, 2 * TT], FP32, name="RMAX", tag="RMAX")
            # psum carries -dist/2 directly (both norm terms live in the
            # matmul), so MA and MB are on the same scale.
            nc.vector.tensor_max(RMAX, MA, MB)
            pss = psf.tile([1, 2 * TT], FP32, name="pss")
            nc.tensor.matmul(pss, lhsT=ones_col, rhs=RMAX, start=True, stop=True)
            ssum = ftmp.tile([1, 1], FP32, name="ssum", tag="ssum")
            nc.vector.tensor_reduce(out=ssum, in_=pss, axis=AX.X, op=OP.add)
            res = ftmp.tile([1, 1], FP32, name="res", tag="res")
            nc.vector.tensor_scalar_mul(res, ssum, -2.0 / N)
            nc.sync.dma_start(out=out, in_=res)


def get_nc():
    if "nc" not in _NC_CACHE:
        _NC_CACHE["nc"] = _build_nc()
    return _NC_CACHE["nc"]


def _host_operands(X):
    """Build the [128, 4096] bf16 operand block pair (L-side, R-side) plus
    the -n per-point bias in [P, TT] layout, for one cloud X [4096, 3]."""
    import ml_dtypes
    BF = ml_dtypes.bfloat16
    # column c = t*128 + p  <->  point p*32 + t
    c = np.arange(N)
    perm = (c % P) * TT + c // P
    xp = np.ascontiguousarray(X[perm].T)          # [3, 4096] fp32, col layout
    levels = []
    r = xp
    for _ in range(3):
        h = r.astype(BF)
        levels.append(h)
        r = r - h.astype(np.float32)
    n = (X[perm] ** 2).sum(1).astype(np.float32)  # [4096] norms, col layout
    nsplit = []
    rn = -0.5 * n
    for _ in range(3):
        h = rn.astype(BF)
        nsplit.append(h)
        rn = rn - h.astype(np.float32)

    def assemble(side):
        blk = np.zeros((32, N), dtype=BF)
        for q, pair in enumerate(COORD_PAIRS):
            blk[3 * q:3 * q + 3] = levels[pair[side]]
        ones = np.ones((3, N), dtype=BF)
        nrows = np.stack(nsplit)
        if side == 0:   # L: ones against R-norms, own norms against R-ones
            blk[18:21] = ones
            blk[21:24] = nrows
        else:
            blk[18:21] = nrows
            blk[21:24] = ones
        return np.tile(blk, (4, 1))               # replicate to 4 quadrants

    return assemble(0), assemble(1)


def kernel(points1, points2, **_ignored):
    from concourse.bass_utils import run_bass_kernel_spmd

    p1 = np.ascontiguousarray(np.asarray(points1, dtype=np.float32))
    p2 = np.ascontiguousarray(np.asarray(points2, dtype=np.float32))
    assert p1.shape == (B, N, D3) and p2.shape == (B, N, D3)

    nc = get_nc()
    in_maps = []
    for b in range(B):
        l0, r0 = _host_operands(p1[b])
        l1, r1 = _host_operands(p2[b])
        in_maps.append({
            "lbuf0": l0, "rbuf0": r0, "lbuf1": l1, "rbuf1": r1,
        })
    res = run_bass_kernel_spmd(nc, in_maps, core_ids=list(range(B)))
    losses = np.array(
        [res.results[b]["loss"][0, 0] for b in range(B)], dtype=np.float32
    )
    return np.float32(losses.mean())


# revision 24
# speedup vs baseline: 1.0101x; 1.0101x over previous
"""Chamfer distance loss kernel for Trainium2 (8 NeuronCores).

Problem: points1, points2 [8, 4096, 3] fp32 -> scalar loss.
Sharding: data-parallel over batch; core b handles batch b. Host averages the
8 per-batch losses.

Per-core algorithm:
  dist[i,j] = n1[i] + n2[j] - 2*x1[i].x2[j]  (squared L2)
  * TensorE: PSUM[i,j] = sum_k L[k,i]*R[k,j] where the 21 live rows are a
    3-level bf16 split of the coordinates (hi/lo/lo2) plus rows carrying
    -n_j/2 (3-level bf16 split), so PSUM = (x_i.x_j)_fp32ish - n_j/2.
    The 21 operand rows (zero-padded to 32) are replicated at partition
    bases 0/32/64/96 and the PE is driven in 4x-row-tiling mode
    (tile_position=(32q, 0)): four K=21 matmuls execute concurrently in
    separate 32-row tiles.  The operand buffers (including the splits,
    norm rows, padding and quadrant replicas) are assembled on the HOST
    and DMAed in directly -- layout/precision prep is O(N), all O(N^2)
    work stays on device -- which removes the on-chip setup phase from
    the critical path.
    Three additional rows carry -n_i/2 (L-side norm splits against R-side
    ones), so PSUM holds -dist/2 directly and no per-row ACT bias is needed.
  * Reduction (engine-balanced; the kernel is ScalarE/VectorE-bound and both
    run ~1 elem/lane/cycle, so work is split to equalize them):
      A-unit (44 of 64): ScalarE ACT Copy psum -> fp16 (-dist/2) for both
        2048-halves, then one VectorE fp16 max-tree (2x mode) over
        [128,4096] + reduce_max -> MB[:, col].
      M-unit (20 of 64): half h0 is reduced STRAIGHT off PSUM by VectorE
        tensor_reduce(max) -> MA[:, col]; half h1 goes through the ScalarE
        cast + a fp16 half-tree -> MB[:, col].  M-units keep ScalarE fed
        (PSUM banks free after 2.3us instead of 4.6us) while soaking up
        VectorE slack.
  * RMAX = max(MA, MB) (same -dist/2 scale); means via ones-vector matmul
    partition-sum scaled by -2/4096.
"""

import numpy as np

N = 4096          # points per cloud
P = 128           # partitions
TT = N // P       # 32 column blocks
D3 = 3
JB = 512          # matmul moving free dim
HALF = N // 2     # per-PSUM-allocation j extent (4 banks)
B = 8             # batches / cores
KPAD = 128
NROWS = 24        # live rows: 18 coord-pair rows + 3 R-norm rows + 3 L-norm rows
N_MIX = 32        # of the 64 (direction, i-tile) units, how many are M-units
NEG_INIT = -1.0e30

# (L-level, R-level) pairs for the 3-level bf16 product expansion.
COORD_PAIRS = [(0, 0), (0, 1), (0, 2), (1, 0), (1, 1), (2, 0)]

_NC_CACHE = {}


def _build_nc():
    import concourse.bacc as bacc
    import concourse.tile as tile
    from concourse import mybir

    FP32 = mybir.dt.float32
    BF16 = mybir.dt.bfloat16

    nc = bacc.Bacc("TRN2", target_bir_lowering=False, debug=False)
    bufs_in = {}
    for name in ("lbuf0", "rbuf1", "lbuf1", "rbuf0"):
        bufs_in[name] = nc.dram_tensor(name, [KPAD, N], BF16,
                                       kind="ExternalInput").ap()
    out = nc.dram_tensor("loss", [1, 1], FP32, kind="ExternalOutput").ap()

    with tile.TileContext(nc) as tc:
        _emit(tc, bufs_in, out)

    nc.compile()
    return nc


def _emit(tc, bufs_in, out):
    import concourse.bass as bass  # noqa: F401
    from concourse import mybir

    FP32 = mybir.dt.float32
    BF16 = mybir.dt.bfloat16
    FP16 = mybir.dt.float16
    AX = mybir.AxisListType
    OP = mybir.AluOpType
    AF = mybir.ActivationFunctionType

    nc = tc.nc

    # Which (direction, i-tile) units are M-units, spread evenly, end-aligned
    # (M-units have less post-matmul latency, shortening the drain).
    n_units = 2 * TT
    mix = {((k + 1) * n_units) // N_MIX - 1 for k in range(N_MIX)}

    from contextlib import ExitStack
    with ExitStack() as ctx:
        consts = ctx.enter_context(tc.tile_pool(name="consts", bufs=1))

        ones_col = consts.tile([P, 1], FP32, name="ones_col", tag="ones_col")
        nc.vector.memset(ones_col, 1.0)

        Lbufs, Rbufs = [], []
        for m in range(2):
            Lb = consts.tile([KPAD, N], BF16, name=f"Lbuf{m}", tag=f"Lbuf{m}")
            Rb = consts.tile([KPAD, N], BF16, name=f"Rbuf{m}", tag=f"Rbuf{m}")
            Lbufs.append(Lb)
            Rbufs.append(Rb)
        MA = consts.tile([P, 2 * TT], FP32, name="MA", tag="MA")
        MB = consts.tile([P, 2 * TT], FP32, name="MB", tag="MB")
        nc.vector.memset(MA, NEG_INIT)

        # HAM warm-up: ~4.3us of back-to-back dummy matmuls while the operand
        # DMAs are in flight.  The PE's activity monitor only unthrottles
        # (1.2 -> 2.4 GHz) after a full 4096-cycle busy window; the main loop
        # alone never fills one (~30% duty), but once warm the <=1.5us gaps
        # between MM groups keep it warm.  Cold matmuls cost 596ns vs ~230ns
        # per warm 4x-tiled group, and the MM turnaround sits on ScalarE's
        # critical path at every PSUM slot handoff.
        warm = consts.tile([32, JB], BF16, name="warm", tag="warm")
        nc.vector.memset(warm, 0.0)

        # Operand loads, chunked so the first units' semaphores fire early
        # (a whole-buffer DMA only signals at full completion ~10us in).
        # Rbuf1 (every unit j-scans all of it) is split 4+4 across the two
        # HWDGE queues; Lbuf0's first chunk covers i-tiles 0..7.
        CH = 512
        for c in (0, 2):
            nc.sync.dma_start(out=Rbufs[1][:, c * CH:(c + 1) * CH],
                              in_=bufs_in["rbuf1"][:, c * CH:(c + 1) * CH])
        for c in (1, 3):
            nc.scalar.dma_start(out=Rbufs[1][:, c * CH:(c + 1) * CH],
                                in_=bufs_in["rbuf1"][:, c * CH:(c + 1) * CH])
        nc.scalar.dma_start(out=Lbufs[0][:, 0:1024],
                            in_=bufs_in["lbuf0"][:, 0:1024])
        for c in range(4, 8):
            (nc.sync if c % 2 else nc.scalar).dma_start(
                out=Rbufs[1][:, c * CH:(c + 1) * CH],
                in_=bufs_in["rbuf1"][:, c * CH:(c + 1) * CH])
        nc.sync.dma_start(out=Lbufs[0][:, 1024:N],
                          in_=bufs_in["lbuf0"][:, 1024:N])
        nc.sync.dma_start(out=Lbufs[1], in_=bufs_in["lbuf1"])
        nc.sync.dma_start(out=Rbufs[0], in_=bufs_in["rbuf0"])

        with tc.tile_pool(name="wps", bufs=1, space="PSUM") as wps:
            wt = wps.tile([P, JB], FP32, name="wt", tag="wt")
            for _ in range(10):
                nc.tensor.matmul(wt, lhsT=warm[:, 0:P], rhs=warm,
                                 start=True, stop=True)

        # ---------------- main loop ----------------
        unit = 0
        with tc.tile_pool(name="psm", bufs=2, space="PSUM") as psm, \
             tc.tile_pool(name="dpool", bufs=2) as dpool, \
             tc.tile_pool(name="mpool", bufs=2) as mpool, \
             tc.tile_pool(name="papool", bufs=2) as papool, \
             tc.tile_pool(name="pbpool", bufs=2) as pbpool, \
             tc.tile_pool(name="mapool", bufs=2) as mapool, \
             tc.tile_pool(name="mbpool", bufs=2) as mbpool:
            def mm_group(ps, Lb, Rb, t, h):
                for u in range(HALF // JB):
                    j0 = h * HALF + u * JB
                    q = 32 * (u % 4)
                    nc.tensor.matmul(
                        ps[:, u * JB:(u + 1) * JB],
                        lhsT=Lb[q:q + NROWS, t * P:(t + 1) * P],
                        rhs=Rb[q:q + NROWS, j0:j0 + JB],
                        start=True, stop=True,
                        tile_position=(q, 0),
                    )

            for d in range(2):
                Lb = Lbufs[0] if d == 0 else Lbufs[1]
                Rb = Rbufs[1] if d == 0 else Rbufs[0]
                for t in range(TT):
                    col = d * TT + t
                    use_m = unit in mix
                    unit += 1
                    if use_m:
                        # --- M-unit: h0 straight off PSUM, h1 cast+half-tree
                        # Late units emit the TR half first so its 2.3us PSUM
                        # reduce overlaps the final casts instead of trailing.
                        late = unit > n_units - 3
                        if late:
                            ps0 = psm.tile([P, HALF], FP32, name="ps", tag="ps")
                            mm_group(ps0, Lb, Rb, t, 0)
                            nc.vector.tensor_reduce(
                                out=MA[:, col:col + 1], in_=ps0,
                                axis=AX.X, op=OP.max)
                        ps1 = psm.tile([P, HALF], FP32, name="ps", tag="ps")
                        mm_group(ps1, Lb, Rb, t, 1)
                        Dm = mpool.tile([P, HALF], FP16, name="Dm", tag="Dm")
                        nc.scalar.copy(Dm, ps1)
                        if not late:
                            ps0 = psm.tile([P, HALF], FP32, name="ps", tag="ps")
                            mm_group(ps0, Lb, Rb, t, 0)
                            nc.vector.tensor_reduce(
                                out=MA[:, col:col + 1], in_=ps0,
                                axis=AX.X, op=OP.max)
                        ma = mapool.tile([P, HALF // 2], FP16, name="ma",
                                         tag="ma")
                        mb = mbpool.tile([P, HALF // 4], FP16, name="mb",
                                         tag="mb")
                        nc.vector.tensor_max(ma, Dm[:, :1024], Dm[:, 1024:])
                        nc.vector.tensor_max(mb, ma[:, :512], ma[:, 512:1024])
                        nc.vector.tensor_reduce(
                            out=MB[:, col:col + 1],
                            in_=mb, axis=AX.X, op=OP.max)
                    else:
                        # --- A-unit: ScalarE cast both halves + fp16 tree ---
                        Dt = dpool.tile([P, N], FP16, name="Dt", tag="Dt")
                        for h in range(2):
                            ps = psm.tile([P, HALF], FP32, name="ps", tag="ps")
                            mm_group(ps, Lb, Rb, t, h)
                            nc.scalar.copy(Dt[:, h * HALF:(h + 1) * HALF], ps)
                        PA = papool.tile([P, HALF], FP16, name="PA", tag="PA")
                        PB = pbpool.tile([P, HALF // 2], FP16,
                                         name="PB", tag="PB")
                        nc.vector.tensor_max(PA, Dt[:, :HALF], Dt[:, HALF:])
                        nc.vector.tensor_max(PB, PA[:, :1024], PA[:, 1024:2048])
                        nc.vector.tensor_max(PA[:, :512], PB[:, :512],
                                             PB[:, 512:1024])
                        nc.vector.tensor_reduce(
                            out=MB[:, col:col + 1],
                            in_=PA[:, :512], axis=AX.X, op=OP.max,
                        )

        # ---------------- fixup + final reduction ----------------
        with tc.tile_pool(name="psf", bufs=1, space="PSUM") as psf, \
             tc.tile_pool(name="ftmp", bufs=1) as ftmp:
            RMAX = ftmp.tile([P, 2 * TT], FP32, name="RMAX", tag="RMAX")
            # psum carries -dist/2 directly (both norm terms live in the
            # matmul), so MA and MB are on the same scale.
            nc.vector.tensor_max(RMAX, MA, MB)
            pss = psf.tile([1, 2 * TT], FP32, name="pss")
            nc.tensor.matmul(pss, lhsT=ones_col, rhs=RMAX, start=True, stop=True)
            ssum = ftmp.tile([1, 1], FP32, name="ssum", tag="ssum")
            nc.vector.tensor_reduce(out=ssum, in_=pss, axis=AX.X, op=OP.add)
            res = ftmp.tile([1, 1], FP32, name="res", tag="res")
            nc.vector.tensor_scalar_mul(res, ssum, -2.0 / N)
            nc.sync.dma_start(out=out, in_=res)


def get_nc():
    if "nc" not in _NC_CACHE:
        _NC_CACHE["nc"] = _build_nc()
    return _NC_CACHE["nc"]


def _host_operands(X):
    """Build the [128, 4096] bf16 operand block pair (L-side, R-side) plus
    the -n per-point bias in [P, TT] layout, for one cloud X [4096, 3]."""
    import ml_dtypes
    BF = ml_dtypes.bfloat16
    # column c = t*128 + p  <->  point p*32 + t
    c = np.arange(N)
    perm = (c % P) * TT + c // P
    xp = np.ascontiguousarray(X[perm].T)          # [3, 4096] fp32, col layout
    levels = []
    r = xp
    for _ in range(3):
        h = r.astype(BF)
        levels.append(h)
        r = r - h.astype(np.float32)
    n = (X[perm] ** 2).sum(1).astype(np.float32)  # [4096] norms, col layout
    nsplit = []
    rn = -0.5 * n
    for _ in range(3):
        h = rn.astype(BF)
        nsplit.append(h)
        rn = rn - h.astype(np.float32)

    def assemble(side):
        blk = np.zeros((32, N), dtype=BF)
        for q, pair in enumerate(COORD_PAIRS):
            blk[3 * q:3 * q + 3] = levels[pair[side]]
        ones = np.ones((3, N), dtype=BF)
        nrows = np.stack(nsplit)
        if side == 0:   # L: ones against R-norms, own norms against R-ones
            blk[18:21] = ones
            blk[21:24] = nrows
        else:
            blk[18:21] = nrows
            blk[21:24] = ones
        return np.tile(blk, (4, 1))               # replicate to 4 quadrants

    return assemble(0), assemble(1)


def kernel(points1, points2, **_ignored):
    from concourse.bass_utils import run_bass_kernel_spmd

    p1 = np.ascontiguousarray(np.asarray(points1, dtype=np.float32))
    p2 = np.ascontiguousarray(np.asarray(points2, dtype=np.float32))
    assert p1.shape == (B, N, D3) and p2.shape == (B, N, D3)

    nc = get_nc()
    in_maps = []
    for b in range(B):
        l0, r0 = _host_operands(p1[b])
        l1, r1 = _host_operands(p2[b])
        in_maps.append({
            "lbuf0": l0, "rbuf0": r0, "lbuf1": l1, "rbuf1": r1,
        })
    res = run_bass_kernel_spmd(nc, in_maps, core_ids=list(range(B)))
    losses = np.array(
        [res.results[b]["loss"][0, 0] for b in range(B)], dtype=np.float32
    )
    return np.float32(losses.mean())
